# revision 26
# baseline (speedup 1.0000x reference)
"""AspectFocusedMetaLearning Trainium2 kernel (8 NeuronCores, SPMD).

Strategy:
  - Data-parallel over query batches: 64 batches -> 8 per core (4096 tokens).
  - Support set sharded: 32 examples -> 4 per core; prototype shards
    AllGathered into the full [32, 768] prototype table.
  - Activations feature-major on chip ([768 -> 6x128 partitions, tokens]);
    weights stationary (lhsT); all matmuls in bf16 (1 cyc/row, FWL), f32 PSUM.
  - Per-token reductions (norms / softmax-sum / LN stats) via ones-vector
    matmuls on the TensorEngine; per-token broadcasts via K=1 rank-1 matmuls.
  - Context 1: support encode + AllGather + prototype normalization feeding
    straight into the adaptation loop, which processes TWO 512-token tiles
    interleaved so one tile's matmul burst hides the other's softmax chain.
  - Context 2: encoders + LN + meta classifier at N=256, all weights resident.
Outputs (matching reference): preds [64,512,3], adapted [64,512,768],
protos [32,768], q_comb [64,512,1536].
"""

import numpy as np

H = 768
P = 128
KC = H // P            # 6 feature chunks
NCORES = 8
QB, QS = 64, 512       # query batches, seq len
SB, SS = 32, 256       # support batches, seq len
TOKQ = QB * QS // NCORES   # 4096 query tokens per core
TOKS = SB * SS // NCORES   # 1024 support tokens per core
NEX = SB // NCORES         # 4 support examples per core
TS_A, NT_A = 512, 8        # adaptation tiles
TS_F, NT_F = 256, 16       # final-phase tiles
ADAPT_STEPS = 5
EPS_LN = 1e-5

LAST_EXEC_NS = None
_CACHE = {}

W2D = {
    "aw_W1": (H, H), "aw_W2": (H, H),
    "ct_W1": (H, H), "ct_W2": (H, H),
    "ad_W1": (H, H), "ad_W2": (H, H),
    "mc_W1": (2 * H, H), "mc_W2": (H, 64), "mc_W3": (64, 3),
}
V1D = {
    "aw_b1": H, "aw_b2": H, "aw_g": H, "aw_be": H,
    "ct_b1": H, "ct_b2": H, "ct_g": H, "ct_be": H,
    "ad_b1": H, "ad_b2": H,
    "mc_b1": H, "mc_b2": 64, "mc_b3": 3,
}


def _build():
    from concourse import bass, bacc, mybir, tile
    from concourse.masks import make_identity

    f32 = mybir.dt.float32
    bf16 = mybir.dt.bfloat16
    AF = mybir.ActivationFunctionType
    AX = mybir.AxisListType

    nc = bacc.Bacc("TRN2", target_bir_lowering=False, debug=False,
                   num_devices=NCORES)

    # ---- DRAM I/O ----
    qe = nc.dram_tensor("qe", [TOKQ, H], f32, kind="ExternalInput").ap()
    se = nc.dram_tensor("se", [TOKS, H], f32, kind="ExternalInput").ap()
    mw = nc.dram_tensor("mw", [1, TOKS], f32, kind="ExternalInput").ap()
    wap = {}
    for name, (k, m) in W2D.items():
        wap[name] = nc.dram_tensor(name + "bf", [k, m], bf16,
                                   kind="ExternalInput").ap()
    for name, m in V1D.items():
        wap[name] = nc.dram_tensor(name, [m], f32, kind="ExternalInput").ap()

    preds_o = nc.dram_tensor("preds_o", [TOKQ, 3], f32, kind="ExternalOutput").ap()
    adapted_o = nc.dram_tensor("adapted_o", [TOKQ, H], f32, kind="ExternalOutput").ap()
    qcomb_o = nc.dram_tensor("qcomb_o", [TOKQ, 2 * H], f32, kind="ExternalOutput").ap()
    protos_o = nc.dram_tensor("protos_o", [SB, H], f32, kind="ExternalOutput").ap()

    afm = nc.dram_tensor("afm", [P, KC, TOKQ], bf16, kind="Internal").ap()

    cnt = [0]
    lp = dict(reason="bf16 PE pipeline")

    def make_ops(tc, sb, ps, cpl, mm_bufs=3, att_bufs=3):
        ones128 = cpl.tile([P, 1], bf16, tag="ones128", name="ones128")
        nc.vector.memset(ones128, 1.0)
        onesbc = cpl.tile([1, P], bf16, tag="onesbc", name="onesbc")
        nc.vector.memset(onesbc, 1.0)
        epsln = cpl.tile([1, 1], f32, tag="epsln", name="epsln")
        nc.vector.memset(epsln, EPS_LN)

        def mm(out, lhsT, rhs, start, stop):
            nc.tensor.matmul(out, lhsT, rhs, start=start, stop=stop)

        def copy_alt(dst, src):
            cnt[0] += 1
            if cnt[0] % 2 == 0:
                nc.scalar.activation(dst, src, AF.Copy)
            else:
                nc.vector.tensor_copy(dst, src)

        def load_w(name, nm=None):
            k, m = W2D[name]
            t = cpl.tile([P, k // P, m], bf16, tag=nm or ("w_" + name),
                         name=nm or ("w_" + name))
            nc.sync.dma_start(t, wap[name].rearrange("(k p) m -> p k m", p=P))
            return t

        def load_v(name):
            m = V1D[name]
            if m >= P:
                t = cpl.tile([P, m // P], f32, tag="b_" + name, name="b_" + name)
                nc.sync.dma_start(t, wap[name].rearrange("(m p) -> p m", p=P))
            else:
                t = cpl.tile([m, 1], f32, tag="b_" + name, name="b_" + name)
                nc.sync.dma_start(t, wap[name].rearrange("(m o) -> m o", o=1))
            return t

        def ffn(w, b, chunks, n, func, out_tag, out_dt=bf16, out_bufs=12):
            kb_n = len(chunks)
            m_n = w.shape[2] // P
            outs = []
            for mb in range(m_n):
                acc = ps.tile([P, n], f32, tag="mm", bufs=mm_bufs, name="accf")
                for kb in range(kb_n):
                    mm(acc, w[:, kb, mb * P:(mb + 1) * P], chunks[kb],
                       kb == 0, kb == kb_n - 1)
                o = sb.tile([P, n], out_dt, tag=out_tag, bufs=out_bufs,
                            name="ffn_o")
                with nc.allow_low_precision(**lp):
                    nc.scalar.activation(o, acc, func, bias=b[:, mb:mb + 1])
                outs.append(o)
            return outs

        def layer_norm(chunks, n, g, be, out_tag, out_dt=f32, out_bufs=6):
            mu_ps = ps.tile([1, n], f32, tag="row", bufs=2, name="mu_ps")
            for c in range(KC):
                mm(mu_ps, ones128, chunks[c], c == 0, c == KC - 1)
            s2_ps = ps.tile([1, n], f32, tag="row", bufs=2, name="s2_ps")
            for c in range(KC):
                sq = sb.tile([P, n], bf16, tag="sq", bufs=4, name="sq")
                nc.vector.tensor_mul(sq, chunks[c], chunks[c])
                mm(s2_ps, ones128, sq, c == 0, c == KC - 1)
            mu = sb.tile([1, n], bf16, tag="rowsb", bufs=8, name="mu")
            with nc.allow_low_precision(**lp):
                nc.scalar.activation(mu, mu_ps, AF.Copy, scale=1.0 / H)
            s2 = sb.tile([1, n], f32, tag="rowsb", bufs=8, name="s2")
            nc.scalar.activation(s2, s2_ps, AF.Copy, scale=1.0 / H)
            var = sb.tile([1, n], f32, tag="rowsb", bufs=8, name="var")
            nc.vector.tensor_mul(var, mu, mu)
            nc.vector.tensor_sub(var, s2, var)
            sd = sb.tile([1, n], f32, tag="rowsb", bufs=8, name="sd")
            nc.scalar.activation(sd, var, AF.Sqrt, bias=epsln[0:1, 0:1])
            rstd = sb.tile([1, n], bf16, tag="rowsb", bufs=8, name="rstd")
            with nc.allow_low_precision(**lp):
                nc.vector.reciprocal(rstd, sd)
            muB = ps.tile([P, n], f32, tag="att", bufs=att_bufs, name="muB")
            mm(muB, onesbc, mu, True, True)
            rstdB = ps.tile([P, n], f32, tag="att", bufs=att_bufs, name="rstdB")
            mm(rstdB, onesbc, rstd, True, True)
            outs = []
            for c in range(KC):
                t = sb.tile([P, n], f32, tag="tmp2", bufs=2, name="lnt")
                nc.vector.tensor_sub(t, chunks[c], muB)
                nc.vector.tensor_mul(t, t, rstdB)
                o = sb.tile([P, n], out_dt, tag=out_tag, bufs=out_bufs,
                            name="ln_o")
                with nc.allow_low_precision(**lp):
                    nc.scalar.activation(o, t, AF.Identity,
                                         bias=be[:, c:c + 1],
                                         scale=g[:, c:c + 1])
                outs.append(o)
            return outs

        def transpose_in(tm_blocks, n, ident_, out_tag="x", out_bufs=26):
            outs = []
            for c in range(KC):
                xc = sb.tile([P, n], bf16, tag=out_tag, bufs=out_bufs,
                             name="xc")
                for b in range(n // P):
                    tr = ps.tile([P, P], f32, tag="mm", bufs=mm_bufs, name="tr")
                    nc.tensor.transpose(
                        tr, tm_blocks[b][:, c * P:(c + 1) * P], ident_)
                    with nc.allow_low_precision(**lp):
                        copy_alt(xc[:, b * P:(b + 1) * P], tr)
                outs.append(xc)
            return outs

        return (mm, copy_alt, load_w, load_v, ffn, layer_norm, transpose_in,
                ones128, onesbc, epsln)

    # ============ Context 1: support + prototypes + adaptation ============
    with tile.TileContext(nc) as tc:
        with tc.tile_pool(name="cp1", bufs=1) as cpl, \
             tc.tile_pool(name="sb1", bufs=2) as sb, \
             tc.tile_pool(name="ps1", bufs=2, space="PSUM") as ps, \
             tc.tile_pool(name="dram1", bufs=1, space="DRAM") as dram:
            (mm, copy_alt, load_w, load_v, ffn, layer_norm, transpose_in,
             ones128, onesbc, epsln) = make_ops(tc, sb, ps, cpl)
            ident = cpl.tile([P, P], f32, tag="ident", name="ident")
            make_identity(nc, ident)
            identb = cpl.tile([P, P], bf16, tag="identb", name="identb")
            make_identity(nc, identb)
            ones32 = cpl.tile([32, 1], bf16, tag="ones32", name="ones32")
            nc.vector.memset(ones32, 1.0)

            aw_W1 = load_w("aw_W1")
            aw_W2 = load_w("aw_W2")
            aw_b1 = load_v("aw_b1"); aw_b2 = load_v("aw_b2")
            aw_g = load_v("aw_g"); aw_be = load_v("aw_be")
            ad_W1 = load_w("ad_W1")
            ad_W2 = load_w("ad_W2")
            ad_b1 = load_v("ad_b1"); ad_b2 = load_v("ad_b2")
            pfm = [cpl.tile([P, NEX], f32, tag=f"pfm{c}", name=f"pfm{c}")
                   for c in range(KC)]

            # ---- support encode + masked mean ----
            for st in range(TOKS // TS_A):
                t0 = st * TS_A
                blocks = []
                for b in range(TS_A // P):
                    blk = sb.tile([P, H], f32, tag="tmin", bufs=10, name="blk")
                    nc.sync.dma_start(blk, se[t0 + b * P: t0 + (b + 1) * P, :])
                    blocks.append(blk)
                xs = transpose_in(blocks, TS_A, ident)
                h1 = ffn(aw_W1, aw_b1, xs, TS_A, AF.Relu, "h1")
                hh = ffn(aw_W2, aw_b2, h1, TS_A, AF.Identity, "inp")
                aware = layer_norm(hh, TS_A, aw_g, aw_be, "aw_s", out_bufs=6)
                mwrowf = sb.tile([1, TS_A], f32, tag="rowsb", bufs=8,
                                 name="mwrf")
                nc.sync.dma_start(mwrowf, mw[:, t0:t0 + TS_A])
                mwrow = sb.tile([1, TS_A], bf16, tag="rowsb", bufs=8,
                                name="mwr")
                with nc.allow_low_precision(**lp):
                    nc.scalar.activation(mwrow, mwrowf, AF.Copy)
                mwB = ps.tile([P, TS_A], f32, tag="att", bufs=3, name="mwB")
                mm(mwB, onesbc, mwrow, True, True)
                for c in range(KC):
                    wt = sb.tile([P, TS_A], f32, tag="tmp2", bufs=2, name="wt")
                    nc.vector.tensor_mul(wt, aware[c], mwB)
                    for e in range(TS_A // SS):
                        eg = st * (TS_A // SS) + e
                        nc.vector.reduce_sum(
                            pfm[c][:, eg:eg + 1],
                            wt[:, e * SS:(e + 1) * SS], axis=AX.X)

            # ---- AllGather prototypes ----
            protos_local = dram.tile([NEX, H], f32, name="protos_local")
            plr = protos_local.rearrange("e (c p) -> p c e", p=P)
            for c in range(KC):
                nc.sync.dma_start(plr[:, c, :], pfm[c])
            protos_all = dram.tile([SB, H], f32, name="protos_all")
            nc.gpsimd.collective_compute(
                "AllGather", mybir.AluOpType.bypass,
                ins=[protos_local.opt()], outs=[protos_all.opt()],
                replica_groups=[list(range(NCORES))])
            nc.sync.dma_start(protos_o, protos_all)

            # ---- normalized prototypes (on-chip) ----
            ptf = cpl.tile([SB, H], f32, tag="ptf", name="ptf")
            nc.sync.dma_start(ptf, protos_all)
            pt = cpl.tile([SB, H], bf16, tag="pt", name="pt")
            with nc.allow_low_precision(**lp):
                nc.scalar.activation(pt, ptf, AF.Copy)
            psq = sb.tile([SB, H], f32, tag="tmin", bufs=10, name="psq")
            nc.vector.tensor_mul(psq, ptf, ptf)
            n2 = sb.tile([SB, 1], f32, tag="small", bufs=4, name="n2")
            nc.vector.reduce_sum(n2, psq, axis=AX.X)
            nrm = sb.tile([SB, 1], f32, tag="small", bufs=4, name="nrm")
            nc.scalar.activation(nrm, n2, AF.Sqrt)
            nc.vector.tensor_scalar_max(nrm, nrm, 1e-8)
            rin = sb.tile([SB, 1], f32, tag="small", bufs=4, name="rin")
            nc.vector.reciprocal(rin, nrm)
            pn_tm = sb.tile([SB, H], f32, tag="tmin", bufs=10, name="pn_tm")
            nc.vector.tensor_scalar_mul(pn_tm, ptf, rin)
            pnf = []
            for c in range(KC):
                tr = ps.tile([P, 32], f32, tag="mm", bufs=3, name="trp")
                nc.tensor.transpose(tr, pn_tm[:, c * P:(c + 1) * P],
                                    ident[0:SB, 0:SB])
                pc = cpl.tile([P, SB], bf16, tag=f"pnf{c}", name=f"pnf{c}")
                with nc.allow_low_precision(**lp):
                    copy_alt(pc, tr)
                pnf.append(pc)

            # ---- adaptation: two tiles interleaved ----
            def adapt_step(x):
                ssum = ps.tile([1, TS_A], f32, tag="row", bufs=2, name="ssum")
                for c in range(KC):
                    sq = sb.tile([P, TS_A], bf16, tag="sq", bufs=4, name="sq")
                    nc.vector.tensor_mul(sq, x[c], x[c])
                    mm(ssum, ones128, sq, c == 0, c == KC - 1)
                simsT = ps.tile([32, TS_A], f32, tag="att", bufs=3,
                                name="simsT")
                for c in range(KC):
                    mm(simsT, pnf[c], x[c], c == 0, c == KC - 1)
                nrmx = sb.tile([1, TS_A], f32, tag="rowsb", bufs=8,
                               name="nrmx")
                nc.scalar.activation(nrmx, ssum, AF.Sqrt)
                rno = sb.tile([1, TS_A], bf16, tag="rowsb", bufs=8, name="rno")
                with nc.allow_low_precision(**lp):
                    nc.vector.reciprocal(rno, nrmx)
                rnoB = ps.tile([32, TS_A], f32, tag="att", bufs=3, name="rnoB")
                mm(rnoB, onesbc[:, 0:SB], rno, True, True)
                rnoBs = sb.tile([32, TS_A], f32, tag="attsb", bufs=8,
                                name="rnoBs")
                nc.vector.tensor_copy(rnoBs, rnoB)
                ssc = sb.tile([32, TS_A], f32, tag="attsb", bufs=8, name="ssc")
                nc.vector.tensor_mul(ssc, simsT, rnoBs)
                E = sb.tile([32, TS_A], bf16, tag="attsb", bufs=8, name="E")
                with nc.allow_low_precision(**lp):
                    nc.scalar.activation(E, ssc, AF.Exp)
                Zs = ps.tile([1, TS_A], f32, tag="row", bufs=2, name="Zs")
                mm(Zs, ones32, E, True, True)
                rz = sb.tile([1, TS_A], bf16, tag="rowsb", bufs=8, name="rz")
                with nc.allow_low_precision(**lp):
                    nc.vector.reciprocal(rz, Zs)
                rzB = ps.tile([32, TS_A], f32, tag="att", bufs=3, name="rzB")
                mm(rzB, onesbc[:, 0:SB], rz, True, True)
                wgt = sb.tile([32, TS_A], bf16, tag="attsb", bufs=8,
                              name="wgt")
                with nc.allow_low_precision(**lp):
                    nc.vector.tensor_mul(wgt, E, rzB)
                inp = []
                for c in range(KC):
                    ifl = ps.tile([P, TS_A], f32, tag="mm", bufs=3, name="ifl")
                    mm(ifl, pt[:, c * P:(c + 1) * P], wgt, True, True)
                    ic = sb.tile([P, TS_A], bf16, tag="inp", bufs=12,
                                 name="ic")
                    with nc.allow_low_precision(**lp):
                        nc.vector.tensor_add(ic, ifl, x[c])
                    inp.append(ic)
                h1 = ffn(ad_W1, ad_b1, inp, TS_A, AF.Relu, "h1")
                return ffn(ad_W2, ad_b2, h1, TS_A, AF.Identity, "x",
                           out_bufs=26)

            for pair in range(NT_A // 2):
                xs2 = []
                for t in range(2):
                    ti = pair * 2 + t
                    t0 = ti * TS_A
                    blocks = []
                    for b in range(TS_A // P):
                        blk = sb.tile([P, H], f32, tag="tmin", bufs=10,
                                      name="blk")
                        nc.sync.dma_start(
                            blk, qe[t0 + b * P: t0 + (b + 1) * P, :])
                        blocks.append(blk)
                    xs2.append(transpose_in(blocks, TS_A, ident))
                for _ in range(ADAPT_STEPS):
                    xs2 = [adapt_step(xs2[0]), adapt_step(xs2[1])]
                for t in range(2):
                    ti = pair * 2 + t
                    t0 = ti * TS_A
                    x = xs2[t]
                    for c in range(KC):
                        nc.sync.dma_start(afm[:, c, t0:t0 + TS_A], x[c])
                    for b in range(TS_A // P):
                        atm = sb.tile([P, H], f32, tag="tmout", bufs=4,
                                      name="atm")
                        for c in range(KC):
                            tr = ps.tile([P, P], bf16, tag="mm", bufs=3,
                                         name="tra")
                            nc.tensor.transpose(
                                tr, x[c][:, b * P:(b + 1) * P], identb)
                            copy_alt(atm[:, c * P:(c + 1) * P], tr)
                        nc.sync.dma_start(
                            adapted_o[t0 + b * P: t0 + (b + 1) * P, :], atm)

    # ============ Context 2: encoders + LN + meta classifier ============
    with tile.TileContext(nc) as tc:
        with tc.tile_pool(name="cp3", bufs=1) as cpl, \
             tc.tile_pool(name="sb3", bufs=2) as sb, \
             tc.tile_pool(name="ps3", bufs=2, space="PSUM") as ps:
            (mm, copy_alt, load_w, load_v, ffn, layer_norm, transpose_in,
             ones128, onesbc, epsln) = make_ops(tc, sb, ps, cpl,
                                                mm_bufs=2, att_bufs=4)
            identb = cpl.tile([P, P], bf16, tag="identb", name="identb")
            make_identity(nc, identb)

            aw_W1f = load_w("aw_W1", "aw_W1f")
            aw_W2f = load_w("aw_W2", "aw_W2f")
            aw_b1 = load_v("aw_b1"); aw_b2 = load_v("aw_b2")
            aw_g = load_v("aw_g"); aw_be = load_v("aw_be")
            ct_W1 = load_w("ct_W1")
            ct_W2 = load_w("ct_W2")
            ct_b1 = load_v("ct_b1"); ct_b2 = load_v("ct_b2")
            ct_g = load_v("ct_g"); ct_be = load_v("ct_be")
            mc_W1t = load_w("mc_W1")
            mc_W2t = cpl.tile([P, KC, 64], bf16, tag="w_mc_W2", name="w_mc_W2")
            nc.sync.dma_start(mc_W2t,
                              wap["mc_W2"].rearrange("(k p) m -> p k m", p=P))
            mc_W3t = cpl.tile([64, 3], bf16, tag="w_mc_W3", name="w_mc_W3")
            nc.sync.dma_start(mc_W3t, wap["mc_W3"])
            mc_b1 = load_v("mc_b1"); mc_b2 = load_v("mc_b2")
            mc_b3 = load_v("mc_b3")

            for tj in range(NT_F):
                t0 = tj * TS_F
                adT = []
                for c in range(KC):
                    a = sb.tile([P, TS_F], bf16, tag="x", bufs=14, name="adT")
                    nc.sync.dma_start(a, afm[:, c, t0:t0 + TS_F])
                    adT.append(a)
                h1a = ffn(aw_W1f, aw_b1, adT, TS_F, AF.Relu, "h1")
                hha = ffn(aw_W2f, aw_b2, h1a, TS_F, AF.Identity, "inp")
                qa = layer_norm(hha, TS_F, aw_g, aw_be, "qln",
                                out_dt=bf16, out_bufs=24)
                h1c = ffn(ct_W1, ct_b1, adT, TS_F, AF.Relu, "h1")
                hhc = ffn(ct_W2, ct_b2, h1c, TS_F, AF.Identity, "inp")
                qc = layer_norm(hhc, TS_F, ct_g, ct_be, "qln",
                                out_dt=bf16, out_bufs=24)

                mh1 = []
                for mb in range(KC):
                    acc = ps.tile([P, TS_F], f32, tag="mm", bufs=2,
                                  name="accm")
                    for kb in range(12):
                        src = qa[kb] if kb < KC else qc[kb - KC]
                        mm(acc, mc_W1t[:, kb, mb * P:(mb + 1) * P], src,
                           kb == 0, kb == 11)
                    o = sb.tile([P, TS_F], bf16, tag="h1", bufs=12, name="mh1")
                    with nc.allow_low_precision(**lp):
                        nc.scalar.activation(o, acc, AF.Relu,
                                             bias=mc_b1[:, mb:mb + 1])
                    mh1.append(o)
                acc2 = ps.tile([64, TS_F], f32, tag="mm", bufs=2, name="acc2")
                for kb in range(KC):
                    mm(acc2, mc_W2t[:, kb, 0:64], mh1[kb], kb == 0,
                       kb == KC - 1)
                h2 = sb.tile([64, TS_F], bf16, tag="h2", bufs=4, name="h2")
                with nc.allow_low_precision(**lp):
                    nc.scalar.activation(h2, acc2, AF.Relu,
                                         bias=mc_b2[:, 0:1])
                acc3 = ps.tile([3, TS_F], f32, tag="row", bufs=2, name="acc3")
                mm(acc3, mc_W3t, h2, True, True)
                pr = sb.tile([3, TS_F], f32, tag="pr", bufs=4, name="pr")
                nc.scalar.activation(pr, acc3, AF.Identity,
                                     bias=mc_b3[:, 0:1])
                nc.sync.dma_start(
                    preds_o[t0:t0 + TS_F, :].rearrange("t r -> r t"), pr)

                for br, qch in enumerate((qa, qc)):
                    for b in range(TS_F // P):
                        half = sb.tile([P, H], f32, tag="tmout", bufs=6,
                                       name="half")
                        for c in range(KC):
                            tr = ps.tile([P, P], bf16, tag="mm", bufs=2,
                                         name="trq")
                            nc.tensor.transpose(
                                tr, qch[c][:, b * P:(b + 1) * P], identb)
                            copy_alt(half[:, c * P:(c + 1) * P], tr)
                        nc.sync.dma_start(
                            qcomb_o[t0 + b * P: t0 + (b + 1) * P,
                                    br * H:(br + 1) * H], half)

    nc.compile()
    return nc


def kernel(support_embeddings, support_labels, query_embeddings, params):
    global LAST_EXEC_NS
    import ml_dtypes
    from concourse import bass_utils

    se = np.ascontiguousarray(np.asarray(support_embeddings), dtype=np.float32)
    sl = np.asarray(support_labels)
    qe = np.ascontiguousarray(np.asarray(query_embeddings), dtype=np.float32)
    pr = {k: np.ascontiguousarray(np.asarray(v), dtype=np.float32)
          for k, v in params.items()}

    mask = (sl > 0).astype(np.float32)
    denom = np.maximum(mask.sum(-1, keepdims=True), 1.0)
    maskw = (mask / denom).astype(np.float32)          # [32, 256]

    if "nc" not in _CACHE:
        _CACHE["nc"] = _build()
    nc = _CACHE["nc"]

    in_maps = []
    for i in range(NCORES):
        m = {
            "qe": np.ascontiguousarray(qe[i * 8:(i + 1) * 8].reshape(TOKQ, H)),
            "se": np.ascontiguousarray(
                se[i * NEX:(i + 1) * NEX].reshape(TOKS, H)),
            "mw": np.ascontiguousarray(
                maskw[i * NEX:(i + 1) * NEX].reshape(1, TOKS)),
        }
        for k in W2D:
            m[k + "bf"] = pr[k].astype(ml_dtypes.bfloat16)
        for k in V1D:
            m[k] = pr[k]
        in_maps.append(m)

    res = bass_utils.run_bass_kernel_spmd(nc, in_maps,
                                          core_ids=list(range(NCORES)))
    LAST_EXEC_NS = res.exec_time_ns
    outs = res.results

    preds = np.concatenate(
        [o["preds_o"].reshape(8, QS, 3) for o in outs], axis=0)
    adapted = np.concatenate(
        [o["adapted_o"].reshape(8, QS, H) for o in outs], axis=0)
    qcomb = np.concatenate(
        [o["qcomb_o"].reshape(8, QS, 2 * H) for o in outs], axis=0)
    protos = outs[0]["protos_o"]
    return preds, adapted, protos, qcomb


# revision 27
# speedup vs baseline: 1.0265x; 1.0265x over previous
"""AspectFocusedMetaLearning Trainium2 kernel (8 NeuronCores, SPMD).

Strategy:
  - Data-parallel over query batches: 64 batches -> 8 per core (4096 tokens).
  - Support set sharded: 32 examples -> 4 per core; prototype shards
    AllGathered into the full [32, 768] prototype table.
  - Activations feature-major on chip ([768 -> 6x128 partitions, tokens]);
    weights stationary (lhsT); all matmuls in bf16 (1 cyc/row, FWL), f32 PSUM.
  - Per-token reductions (norms / softmax-sum / LN stats) via ones-vector
    matmuls on the TensorEngine; per-token broadcasts via K=1 rank-1 matmuls.
  - Context 1: support encode + AllGather + prototype normalization feeding
    straight into the adaptation loop, which processes TWO 512-token tiles
    interleaved so one tile's matmul burst hides the other's softmax chain.
  - Context 2: encoders + LN + meta classifier at N=256, all weights resident.
Outputs (matching reference): preds [64,512,3], adapted [64,512,768],
protos [32,768], q_comb [64,512,1536].
"""

import numpy as np

H = 768
P = 128
KC = H // P            # 6 feature chunks
NCORES = 8
QB, QS = 64, 512       # query batches, seq len
SB, SS = 32, 256       # support batches, seq len
TOKQ = QB * QS // NCORES   # 4096 query tokens per core
TOKS = SB * SS // NCORES   # 1024 support tokens per core
NEX = SB // NCORES         # 4 support examples per core
TS_A, NT_A = 512, 8        # adaptation tiles
TS_F, NT_F = 256, 16       # final-phase tiles
ADAPT_STEPS = 5
EPS_LN = 1e-5

LAST_EXEC_NS = None
_CACHE = {}

W2D = {
    "aw_W1": (H, H), "aw_W2": (H, H),
    "ct_W1": (H, H), "ct_W2": (H, H),
    "ad_W1": (H, H), "ad_W2": (H, H),
    "mc_W1": (2 * H, H), "mc_W2": (H, 64), "mc_W3": (64, 3),
}
V1D = {
    "aw_b1": H, "aw_b2": H, "aw_g": H, "aw_be": H,
    "ct_b1": H, "ct_b2": H, "ct_g": H, "ct_be": H,
    "ad_b1": H, "ad_b2": H,
    "mc_b1": H, "mc_b2": 64, "mc_b3": 3,
}


def _build():
    from concourse import bass, bacc, mybir, tile
    from concourse.masks import make_identity

    f32 = mybir.dt.float32
    bf16 = mybir.dt.bfloat16
    AF = mybir.ActivationFunctionType
    AX = mybir.AxisListType

    nc = bacc.Bacc("TRN2", target_bir_lowering=False, debug=False,
                   num_devices=NCORES)

    # ---- DRAM I/O ----
    qe = nc.dram_tensor("qe", [TOKQ, H], f32, kind="ExternalInput").ap()
    se = nc.dram_tensor("se", [TOKS, H], f32, kind="ExternalInput").ap()
    mw = nc.dram_tensor("mw", [1, TOKS], f32, kind="ExternalInput").ap()
    wap = {}
    for name, (k, m) in W2D.items():
        wap[name] = nc.dram_tensor(name + "bf", [k, m], bf16,
                                   kind="ExternalInput").ap()
    for name, m in V1D.items():
        wap[name] = nc.dram_tensor(name, [m], f32, kind="ExternalInput").ap()

    preds_o = nc.dram_tensor("preds_o", [TOKQ, 3], f32, kind="ExternalOutput").ap()
    adapted_o = nc.dram_tensor("adapted_o", [TOKQ, H], f32, kind="ExternalOutput").ap()
    qcomb_o = nc.dram_tensor("qcomb_o", [TOKQ, 2 * H], f32, kind="ExternalOutput").ap()
    protos_o = nc.dram_tensor("protos_o", [SB, H], f32, kind="ExternalOutput").ap()

    afm = nc.dram_tensor("afm", [P, KC, TOKQ], bf16, kind="Internal").ap()

    cnt = [0]
    lp = dict(reason="bf16 PE pipeline")

    def make_ops(tc, sb, ps, cpl, mm_bufs=3, att_bufs=3):
        ones128 = cpl.tile([P, 1], bf16, tag="ones128", name="ones128")
        nc.vector.memset(ones128, 1.0)
        onesbc = cpl.tile([1, P], bf16, tag="onesbc", name="onesbc")
        nc.vector.memset(onesbc, 1.0)
        epsln = cpl.tile([1, 1], f32, tag="epsln", name="epsln")
        nc.vector.memset(epsln, EPS_LN)

        def mm(out, lhsT, rhs, start, stop):
            nc.tensor.matmul(out, lhsT, rhs, start=start, stop=stop)

        def copy_alt(dst, src):
            cnt[0] += 1
            if cnt[0] % 2 == 0:
                nc.scalar.activation(dst, src, AF.Copy)
            else:
                nc.vector.tensor_copy(dst, src)

        def load_w(name, nm=None):
            k, m = W2D[name]
            t = cpl.tile([P, k // P, m], bf16, tag=nm or ("w_" + name),
                         name=nm or ("w_" + name))
            nc.sync.dma_start(t, wap[name].rearrange("(k p) m -> p k m", p=P))
            return t

        def load_v(name):
            m = V1D[name]
            if m >= P:
                t = cpl.tile([P, m // P], f32, tag="b_" + name, name="b_" + name)
                nc.sync.dma_start(t, wap[name].rearrange("(m p) -> p m", p=P))
            else:
                t = cpl.tile([m, 1], f32, tag="b_" + name, name="b_" + name)
                nc.sync.dma_start(t, wap[name].rearrange("(m o) -> m o", o=1))
            return t

        def ffn(w, b, chunks, n, func, out_tag, out_dt=bf16, out_bufs=12):
            kb_n = len(chunks)
            m_n = w.shape[2] // P
            outs = []
            for mb in range(m_n):
                acc = ps.tile([P, n], f32, tag="mm", bufs=mm_bufs, name="accf")
                for kb in range(kb_n):
                    mm(acc, w[:, kb, mb * P:(mb + 1) * P], chunks[kb],
                       kb == 0, kb == kb_n - 1)
                o = sb.tile([P, n], out_dt, tag=out_tag, bufs=out_bufs,
                            name="ffn_o")
                with nc.allow_low_precision(**lp):
                    nc.scalar.activation(o, acc, func, bias=b[:, mb:mb + 1])
                outs.append(o)
            return outs

        def layer_norm(chunks, n, g, be, out_tag, out_dt=f32, out_bufs=6):
            mu_ps = ps.tile([1, n], f32, tag="row", bufs=2, name="mu_ps")
            for c in range(KC):
                mm(mu_ps, ones128, chunks[c], c == 0, c == KC - 1)
            s2_ps = ps.tile([1, n], f32, tag="row", bufs=2, name="s2_ps")
            for c in range(KC):
                sq = sb.tile([P, n], bf16, tag="sq", bufs=4, name="sq")
                nc.vector.tensor_mul(sq, chunks[c], chunks[c])
                mm(s2_ps, ones128, sq, c == 0, c == KC - 1)
            mu = sb.tile([1, n], bf16, tag="rowsb", bufs=8, name="mu")
            with nc.allow_low_precision(**lp):
                nc.scalar.activation(mu, mu_ps, AF.Copy, scale=1.0 / H)
            s2 = sb.tile([1, n], f32, tag="rowsb", bufs=8, name="s2")
            nc.scalar.activation(s2, s2_ps, AF.Copy, scale=1.0 / H)
            var = sb.tile([1, n], f32, tag="rowsb", bufs=8, name="var")
            nc.vector.tensor_mul(var, mu, mu)
            nc.vector.tensor_sub(var, s2, var)
            sd = sb.tile([1, n], f32, tag="rowsb", bufs=8, name="sd")
            nc.scalar.activation(sd, var, AF.Sqrt, bias=epsln[0:1, 0:1])
            rstd = sb.tile([1, n], bf16, tag="rowsb", bufs=8, name="rstd")
            with nc.allow_low_precision(**lp):
                nc.vector.reciprocal(rstd, sd)
            muB = ps.tile([P, n], f32, tag="att", bufs=att_bufs, name="muB")
            mm(muB, onesbc, mu, True, True)
            rstdB = ps.tile([P, n], f32, tag="att", bufs=att_bufs, name="rstdB")
            mm(rstdB, onesbc, rstd, True, True)
            outs = []
            for c in range(KC):
                t = sb.tile([P, n], f32, tag="tmp2", bufs=2, name="lnt")
                nc.vector.tensor_sub(t, chunks[c], muB)
                nc.vector.tensor_mul(t, t, rstdB)
                o = sb.tile([P, n], out_dt, tag=out_tag, bufs=out_bufs,
                            name="ln_o")
                with nc.allow_low_precision(**lp):
                    nc.scalar.activation(o, t, AF.Identity,
                                         bias=be[:, c:c + 1],
                                         scale=g[:, c:c + 1])
                outs.append(o)
            return outs

        def transpose_in(tm_blocks, n, ident_, out_tag="x", out_bufs=26):
            outs = []
            for c in range(KC):
                xc = sb.tile([P, n], bf16, tag=out_tag, bufs=out_bufs,
                             name="xc")
                for b in range(n // P):
                    tr = ps.tile([P, P], f32, tag="mm", bufs=mm_bufs, name="tr")
                    nc.tensor.transpose(
                        tr, tm_blocks[b][:, c * P:(c + 1) * P], ident_)
                    with nc.allow_low_precision(**lp):
                        copy_alt(xc[:, b * P:(b + 1) * P], tr)
                outs.append(xc)
            return outs

        return (mm, copy_alt, load_w, load_v, ffn, layer_norm, transpose_in,
                ones128, onesbc, epsln)

    # ============ Context 1: support + prototypes + adaptation ============
    with tile.TileContext(nc) as tc:
        with tc.tile_pool(name="cp1", bufs=1) as cpl, \
             tc.tile_pool(name="sb1", bufs=2) as sb, \
             tc.tile_pool(name="ps1", bufs=2, space="PSUM") as ps, \
             tc.tile_pool(name="dram1", bufs=1, space="DRAM") as dram:
            (mm, copy_alt, load_w, load_v, ffn, layer_norm, transpose_in,
             ones128, onesbc, epsln) = make_ops(tc, sb, ps, cpl)
            ident = cpl.tile([P, P], f32, tag="ident", name="ident")
            make_identity(nc, ident)
            identb = cpl.tile([P, P], bf16, tag="identb", name="identb")
            make_identity(nc, identb)
            ones32 = cpl.tile([32, 1], bf16, tag="ones32", name="ones32")
            nc.vector.memset(ones32, 1.0)

            aw_W1 = load_w("aw_W1")
            aw_W2 = load_w("aw_W2")
            aw_b1 = load_v("aw_b1"); aw_b2 = load_v("aw_b2")
            aw_g = load_v("aw_g"); aw_be = load_v("aw_be")
            ad_W1 = load_w("ad_W1")
            ad_W2 = load_w("ad_W2")
            ad_b1 = load_v("ad_b1"); ad_b2 = load_v("ad_b2")
            pfm = [cpl.tile([P, NEX], f32, tag=f"pfm{c}", name=f"pfm{c}")
                   for c in range(KC)]

            # ---- support encode + masked mean ----
            for st in range(TOKS // TS_A):
                t0 = st * TS_A
                blocks = []
                for b in range(TS_A // P):
                    blk = sb.tile([P, H], f32, tag="tmin", bufs=10, name="blk")
                    nc.sync.dma_start(blk, se[t0 + b * P: t0 + (b + 1) * P, :])
                    blocks.append(blk)
                xs = transpose_in(blocks, TS_A, ident)
                h1 = ffn(aw_W1, aw_b1, xs, TS_A, AF.Relu, "h1")
                hh = ffn(aw_W2, aw_b2, h1, TS_A, AF.Identity, "inp")
                aware = layer_norm(hh, TS_A, aw_g, aw_be, "aw_s", out_bufs=6)
                mwrowf = sb.tile([1, TS_A], f32, tag="rowsb", bufs=8,
                                 name="mwrf")
                nc.sync.dma_start(mwrowf, mw[:, t0:t0 + TS_A])
                mwrow = sb.tile([1, TS_A], bf16, tag="rowsb", bufs=8,
                                name="mwr")
                with nc.allow_low_precision(**lp):
                    nc.scalar.activation(mwrow, mwrowf, AF.Copy)
                mwB = ps.tile([P, TS_A], f32, tag="att", bufs=3, name="mwB")
                mm(mwB, onesbc, mwrow, True, True)
                for c in range(KC):
                    wt = sb.tile([P, TS_A], f32, tag="tmp2", bufs=2, name="wt")
                    nc.vector.tensor_mul(wt, aware[c], mwB)
                    for e in range(TS_A // SS):
                        eg = st * (TS_A // SS) + e
                        nc.vector.reduce_sum(
                            pfm[c][:, eg:eg + 1],
                            wt[:, e * SS:(e + 1) * SS], axis=AX.X)

            # ---- AllGather prototypes ----
            protos_local = dram.tile([NEX, H], f32, name="protos_local")
            plr = protos_local.rearrange("e (c p) -> p c e", p=P)
            for c in range(KC):
                nc.sync.dma_start(plr[:, c, :], pfm[c])
            protos_all = dram.tile([SB, H], f32, name="protos_all")
            nc.gpsimd.collective_compute(
                "AllGather", mybir.AluOpType.bypass,
                ins=[protos_local.opt()], outs=[protos_all.opt()],
                replica_groups=[list(range(NCORES))])
            nc.sync.dma_start(protos_o, protos_all)

            # ---- normalized prototypes (on-chip) ----
            ptf = cpl.tile([SB, H], f32, tag="ptf", name="ptf")
            nc.sync.dma_start(ptf, protos_all)
            pt = cpl.tile([SB, H], bf16, tag="pt", name="pt")
            with nc.allow_low_precision(**lp):
                nc.scalar.activation(pt, ptf, AF.Copy)
            psq = sb.tile([SB, H], f32, tag="tmin", bufs=10, name="psq")
            nc.vector.tensor_mul(psq, ptf, ptf)
            n2 = sb.tile([SB, 1], f32, tag="small", bufs=4, name="n2")
            nc.vector.reduce_sum(n2, psq, axis=AX.X)
            nrm = sb.tile([SB, 1], f32, tag="small", bufs=4, name="nrm")
            nc.scalar.activation(nrm, n2, AF.Sqrt)
            nc.vector.tensor_scalar_max(nrm, nrm, 1e-8)
            rin = sb.tile([SB, 1], f32, tag="small", bufs=4, name="rin")
            nc.vector.reciprocal(rin, nrm)
            pn_tm = sb.tile([SB, H], f32, tag="tmin", bufs=10, name="pn_tm")
            nc.vector.tensor_scalar_mul(pn_tm, ptf, rin)
            pnf = []
            for c in range(KC):
                tr = ps.tile([P, 32], f32, tag="mm", bufs=3, name="trp")
                nc.tensor.transpose(tr, pn_tm[:, c * P:(c + 1) * P],
                                    ident[0:SB, 0:SB])
                pc = cpl.tile([P, SB], bf16, tag=f"pnf{c}", name=f"pnf{c}")
                with nc.allow_low_precision(**lp):
                    copy_alt(pc, tr)
                pnf.append(pc)

            # ---- adaptation: two tiles interleaved ----
            def adapt_step(x):
                ssum = ps.tile([1, TS_A], f32, tag="row", bufs=2, name="ssum")
                for c in range(KC):
                    sq = sb.tile([P, TS_A], bf16, tag="sq", bufs=4, name="sq")
                    nc.vector.tensor_mul(sq, x[c], x[c])
                    mm(ssum, ones128, sq, c == 0, c == KC - 1)
                simsT = ps.tile([32, TS_A], f32, tag="att", bufs=3,
                                name="simsT")
                for c in range(KC):
                    mm(simsT, pnf[c], x[c], c == 0, c == KC - 1)
                nrmx = sb.tile([1, TS_A], f32, tag="rowsb", bufs=8,
                               name="nrmx")
                nc.scalar.activation(nrmx, ssum, AF.Sqrt)
                rno = sb.tile([1, TS_A], bf16, tag="rowsb", bufs=8, name="rno")
                with nc.allow_low_precision(**lp):
                    nc.vector.reciprocal(rno, nrmx)
                rnoB = ps.tile([32, TS_A], f32, tag="att", bufs=3, name="rnoB")
                mm(rnoB, onesbc[:, 0:SB], rno, True, True)
                rnoBs = sb.tile([32, TS_A], f32, tag="attsb", bufs=8,
                                name="rnoBs")
                nc.vector.tensor_copy(rnoBs, rnoB)
                ssc = sb.tile([32, TS_A], f32, tag="attsb", bufs=8, name="ssc")
                nc.vector.tensor_mul(ssc, simsT, rnoBs)
                E = sb.tile([32, TS_A], bf16, tag="attsb", bufs=8, name="E")
                with nc.allow_low_precision(**lp):
                    nc.scalar.activation(E, ssc, AF.Exp)
                Zs = ps.tile([1, TS_A], f32, tag="row", bufs=2, name="Zs")
                mm(Zs, ones32, E, True, True)
                rz = sb.tile([1, TS_A], bf16, tag="rowsb", bufs=8, name="rz")
                with nc.allow_low_precision(**lp):
                    nc.vector.reciprocal(rz, Zs)
                rzB = ps.tile([32, TS_A], f32, tag="att", bufs=3, name="rzB")
                mm(rzB, onesbc[:, 0:SB], rz, True, True)
                wgt = sb.tile([32, TS_A], bf16, tag="attsb", bufs=8,
                              name="wgt")
                with nc.allow_low_precision(**lp):
                    nc.vector.tensor_mul(wgt, E, rzB)
                inp = []
                for c in range(KC):
                    ifl = ps.tile([P, TS_A], f32, tag="mm", bufs=3, name="ifl")
                    mm(ifl, pt[:, c * P:(c + 1) * P], wgt, True, True)
                    ic = sb.tile([P, TS_A], bf16, tag="inp", bufs=12,
                                 name="ic")
                    with nc.allow_low_precision(**lp):
                        nc.vector.tensor_add(ic, ifl, x[c])
                    inp.append(ic)
                h1 = ffn(ad_W1, ad_b1, inp, TS_A, AF.Relu, "h1")
                return ffn(ad_W2, ad_b2, h1, TS_A, AF.Identity, "x",
                           out_bufs=26)

            for pair in range(NT_A // 2):
                xs2 = []
                for t in range(2):
                    ti = pair * 2 + t
                    t0 = ti * TS_A
                    blocks = []
                    for b in range(TS_A // P):
                        blk = sb.tile([P, H], f32, tag="tmin", bufs=10,
                                      name="blk")
                        nc.sync.dma_start(
                            blk, qe[t0 + b * P: t0 + (b + 1) * P, :])
                        blocks.append(blk)
                    xs2.append(transpose_in(blocks, TS_A, ident))
                for _ in range(ADAPT_STEPS):
                    xs2 = [adapt_step(xs2[0]), adapt_step(xs2[1])]
                for t in range(2):
                    ti = pair * 2 + t
                    t0 = ti * TS_A
                    x = xs2[t]
                    for c in range(KC):
                        nc.sync.dma_start(afm[:, c, t0:t0 + TS_A], x[c])
                    for b in range(TS_A // P):
                        atm = sb.tile([P, H], f32, tag="tmout", bufs=4,
                                      name="atm")
                        for c in range(KC):
                            tr = ps.tile([P, P], bf16, tag="mm", bufs=3,
                                         name="tra")
                            nc.tensor.transpose(
                                tr, x[c][:, b * P:(b + 1) * P], identb)
                            copy_alt(atm[:, c * P:(c + 1) * P], tr)
                        nc.sync.dma_start(
                            adapted_o[t0 + b * P: t0 + (b + 1) * P, :], atm)

    # ============ Context 2: encoders + LN + meta classifier ============
    with tile.TileContext(nc) as tc:
        with tc.tile_pool(name="cp3", bufs=1) as cpl, \
             tc.tile_pool(name="sb3", bufs=2) as sb, \
             tc.tile_pool(name="ps3", bufs=2, space="PSUM") as ps:
            (mm, copy_alt, load_w, load_v, ffn, layer_norm, transpose_in,
             ones128, onesbc, epsln) = make_ops(tc, sb, ps, cpl)
            identb = cpl.tile([P, P], bf16, tag="identb", name="identb")
            make_identity(nc, identb)

            aw_W1f = load_w("aw_W1", "aw_W1f")
            aw_W2f = load_w("aw_W2", "aw_W2f")
            aw_b1 = load_v("aw_b1"); aw_b2 = load_v("aw_b2")
            aw_g = load_v("aw_g"); aw_be = load_v("aw_be")
            ct_W1 = load_w("ct_W1")
            ct_W2 = load_w("ct_W2")
            ct_b1 = load_v("ct_b1"); ct_b2 = load_v("ct_b2")
            ct_g = load_v("ct_g"); ct_be = load_v("ct_be")
            mc_W1t = load_w("mc_W1")
            mc_W2t = cpl.tile([P, KC, 64], bf16, tag="w_mc_W2", name="w_mc_W2")
            nc.sync.dma_start(mc_W2t,
                              wap["mc_W2"].rearrange("(k p) m -> p k m", p=P))
            mc_W3t = cpl.tile([64, 3], bf16, tag="w_mc_W3", name="w_mc_W3")
            nc.sync.dma_start(mc_W3t, wap["mc_W3"])
            mc_b1 = load_v("mc_b1"); mc_b2 = load_v("mc_b2")
            mc_b3 = load_v("mc_b3")

            for tj in range(NT_F):
                t0 = tj * TS_F
                adT = []
                for c in range(KC):
                    a = sb.tile([P, TS_F], bf16, tag="x", bufs=14, name="adT")
                    nc.sync.dma_start(a, afm[:, c, t0:t0 + TS_F])
                    adT.append(a)
                h1a = ffn(aw_W1f, aw_b1, adT, TS_F, AF.Relu, "h1")
                hha = ffn(aw_W2f, aw_b2, h1a, TS_F, AF.Identity, "inp")
                qa = layer_norm(hha, TS_F, aw_g, aw_be, "qln",
                                out_dt=bf16, out_bufs=24)
                h1c = ffn(ct_W1, ct_b1, adT, TS_F, AF.Relu, "h1")
                hhc = ffn(ct_W2, ct_b2, h1c, TS_F, AF.Identity, "inp")
                qc = layer_norm(hhc, TS_F, ct_g, ct_be, "qln",
                                out_dt=bf16, out_bufs=24)

                mh1 = []
                for mb in range(KC):
                    acc = ps.tile([P, TS_F], f32, tag="mm", bufs=3,
                                  name="accm")
                    for kb in range(12):
                        src = qa[kb] if kb < KC else qc[kb - KC]
                        mm(acc, mc_W1t[:, kb, mb * P:(mb + 1) * P], src,
                           kb == 0, kb == 11)
                    o = sb.tile([P, TS_F], bf16, tag="h1", bufs=12, name="mh1")
                    with nc.allow_low_precision(**lp):
                        nc.scalar.activation(o, acc, AF.Relu,
                                             bias=mc_b1[:, mb:mb + 1])
                    mh1.append(o)
                acc2 = ps.tile([64, TS_F], f32, tag="mm", bufs=3, name="acc2")
                for kb in range(KC):
                    mm(acc2, mc_W2t[:, kb, 0:64], mh1[kb], kb == 0,
                       kb == KC - 1)
                h2 = sb.tile([64, TS_F], bf16, tag="h2", bufs=4, name="h2")
                with nc.allow_low_precision(**lp):
                    nc.scalar.activation(h2, acc2, AF.Relu,
                                         bias=mc_b2[:, 0:1])
                acc3 = ps.tile([3, TS_F], f32, tag="row", bufs=2, name="acc3")
                mm(acc3, mc_W3t, h2, True, True)
                pr = sb.tile([3, TS_F], f32, tag="pr", bufs=4, name="pr")
                nc.scalar.activation(pr, acc3, AF.Identity,
                                     bias=mc_b3[:, 0:1])
                nc.sync.dma_start(
                    preds_o[t0:t0 + TS_F, :].rearrange("t r -> r t"), pr)

                for br, qch in enumerate((qa, qc)):
                    for b in range(TS_F // P):
                        half = sb.tile([P, H], f32, tag="tmout", bufs=6,
                                       name="half")
                        for c in range(KC):
                            tr = ps.tile([P, P], bf16, tag="mm", bufs=3,
                                         name="trq")
                            nc.tensor.transpose(
                                tr, qch[c][:, b * P:(b + 1) * P], identb)
                            copy_alt(half[:, c * P:(c + 1) * P], tr)
                        nc.sync.dma_start(
                            qcomb_o[t0 + b * P: t0 + (b + 1) * P,
                                    br * H:(br + 1) * H], half)

    nc.compile()
    return nc


def kernel(support_embeddings, support_labels, query_embeddings, params):
    global LAST_EXEC_NS
    import ml_dtypes
    from concourse import bass_utils

    se = np.ascontiguousarray(np.asarray(support_embeddings), dtype=np.float32)
    sl = np.asarray(support_labels)
    qe = np.ascontiguousarray(np.asarray(query_embeddings), dtype=np.float32)
    pr = {k: np.ascontiguousarray(np.asarray(v), dtype=np.float32)
          for k, v in params.items()}

    mask = (sl > 0).astype(np.float32)
    denom = np.maximum(mask.sum(-1, keepdims=True), 1.0)
    maskw = (mask / denom).astype(np.float32)          # [32, 256]

    if "nc" not in _CACHE:
        _CACHE["nc"] = _build()
    nc = _CACHE["nc"]

    in_maps = []
    for i in range(NCORES):
        m = {
            "qe": np.ascontiguousarray(qe[i * 8:(i + 1) * 8].reshape(TOKQ, H)),
            "se": np.ascontiguousarray(
                se[i * NEX:(i + 1) * NEX].reshape(TOKS, H)),
            "mw": np.ascontiguousarray(
                maskw[i * NEX:(i + 1) * NEX].reshape(1, TOKS)),
        }
        for k in W2D:
            m[k + "bf"] = pr[k].astype(ml_dtypes.bfloat16)
        for k in V1D:
            m[k] = pr[k]
        in_maps.append(m)

    res = bass_utils.run_bass_kernel_spmd(nc, in_maps,
                                          core_ids=list(range(NCORES)))
    LAST_EXEC_NS = res.exec_time_ns
    outs = res.results

    preds = np.concatenate(
        [o["preds_o"].reshape(8, QS, 3) for o in outs], axis=0)
    adapted = np.concatenate(
        [o["adapted_o"].reshape(8, QS, H) for o in outs], axis=0)
    qcomb = np.concatenate(
        [o["qcomb_o"].reshape(8, QS, 2 * H) for o in outs], axis=0)
    protos = outs[0]["protos_o"]
    return preds, adapted, protos, qcomb


# revision 53
# speedup vs baseline: 1.1273x; 1.0982x over previous
"""AspectFocusedMetaLearning Trainium2 kernel (8 NeuronCores, SPMD).

Strategy:
  - Data-parallel over query batches: 64 batches -> 8 per core (4096 tokens).
  - Support set sharded: 32 examples -> 4 per core; prototype shards
    AllGathered into the full [32, 768] prototype table.
  - Activations feature-major on chip ([768 -> 6x128 partitions, tokens]);
    weights stationary (lhsT); all matmuls in bf16 (1 cyc/row, FWL), f32 PSUM.
  - Per-token reductions (norms / softmax-sum / LN stats) via ones-vector
    matmuls on the TensorEngine; per-token broadcasts via K=1 rank-1 matmuls.
  - Context 1: support encode + AllGather + prototype normalization feeding
    straight into the adaptation loop, which processes TWO 512-token tiles
    interleaved so one tile's matmul burst hides the other's softmax chain.
  - Context 2: encoders + LN + meta classifier at N=256, all weights resident.
Outputs (matching reference): preds [64,512,3], adapted [64,512,768],
protos [32,768], q_comb [64,512,1536].
"""

import numpy as np

H = 768
P = 128
KC = H // P            # 6 feature chunks
NCORES = 8
QB, QS = 64, 512       # query batches, seq len
SB, SS = 32, 256       # support batches, seq len
TOKQ = QB * QS // NCORES   # 4096 query tokens per core
TOKS = SB * SS // NCORES   # 1024 support tokens per core
NEX = SB // NCORES         # 4 support examples per core
TS_A, NT_A = 512, 8        # adaptation tiles
TS_F, NT_F = 512, 8       # final-phase tiles
ADAPT_STEPS = 5
EPS_LN = 1e-5

LAST_EXEC_NS = None
_CACHE = {}

W2D = {
    "aw_W1": (H, H), "aw_W2": (H, H),
    "ct_W1": (H, H), "ct_W2": (H, H),
    "ad_W1": (H, H), "ad_W2": (H, H),
    "mc_W1": (2 * H, H), "mc_W2": (H, 64), "mc_W3": (64, 3),
}
V1D = {
    "aw_b1": H, "aw_b2": H, "aw_g": H, "aw_be": H,
    "ct_b1": H, "ct_b2": H, "ct_g": H, "ct_be": H,
    "ad_b1": H, "ad_b2": H,
    "mc_b1": H, "mc_b2": 64, "mc_b3": 3,
}


def _build():
    from concourse import bass, bacc, mybir, tile
    from concourse import bass_isa
    from concourse.masks import make_identity

    f32 = mybir.dt.float32
    bf16 = mybir.dt.bfloat16
    AF = mybir.ActivationFunctionType
    AX = mybir.AxisListType

    nc = bacc.Bacc("TRN2", target_bir_lowering=False, debug=False,
                   num_devices=NCORES)

    # ---- DRAM I/O ----
    qe = nc.dram_tensor("qebf", [TOKQ, H], bf16, kind="ExternalInput").ap()
    se = nc.dram_tensor("sebf", [TOKS, H], bf16, kind="ExternalInput").ap()
    mw = nc.dram_tensor("mw", [1, TOKS], f32, kind="ExternalInput").ap()
    wap = {}
    for name, (k, m) in W2D.items():
        wap[name] = nc.dram_tensor(name + "bf", [k, m], bf16,
                                   kind="ExternalInput").ap()
    for name, m in V1D.items():
        wap[name] = nc.dram_tensor(name, [m], f32, kind="ExternalInput").ap()

    preds_o = nc.dram_tensor("preds_o", [TOKQ, 3], f32, kind="ExternalOutput").ap()
    adapted_o = nc.dram_tensor("adapted_o", [TOKQ, H], f32, kind="ExternalOutput").ap()
    qcomb_o = nc.dram_tensor("qcomb_o", [TOKQ, 2 * H], f32, kind="ExternalOutput").ap()
    protos_o = nc.dram_tensor("protos_o", [SB, H], f32, kind="ExternalOutput").ap()

    afm = nc.dram_tensor("afm", [P, KC, TOKQ], bf16, kind="Internal").ap()

    cnt = [0]
    lp = dict(reason="bf16 PE pipeline")

    def make_ops(tc, sb, ps, cpl, mm_bufs=3, att_bufs=3):
        onesW = cpl.tile([P, P], bf16, tag="onesW", name="onesW")
        nc.vector.memset(onesW, 1.0)
        onesbc = cpl.tile([1, P], bf16, tag="onesbc", name="onesbc")
        nc.vector.memset(onesbc, 1.0)
        epsln = cpl.tile([P, 1], f32, tag="epsln", name="epsln")
        nc.vector.memset(epsln, EPS_LN)

        def mm(out, lhsT, rhs, start, stop):
            nc.tensor.matmul(out, lhsT, rhs, start=start, stop=stop)

        def copy_alt(dst, src):
            cnt[0] += 1
            if cnt[0] % 2 == 0:
                nc.scalar.activation(dst, src, AF.Copy)
            else:
                nc.vector.tensor_copy(dst, src)

        def load_w(name, nm=None):
            k, m = W2D[name]
            t = cpl.tile([P, k // P, m], bf16, tag=nm or ("w_" + name),
                         name=nm or ("w_" + name))
            nc.sync.dma_start(t, wap[name].rearrange("(k p) m -> p k m", p=P))
            return t

        def load_v(name):
            m = V1D[name]
            if m >= P:
                t = cpl.tile([P, m // P], f32, tag="b_" + name, name="b_" + name)
                nc.sync.dma_start(t, wap[name].rearrange("(m p) -> p m", p=P))
            else:
                t = cpl.tile([m, 1], f32, tag="b_" + name, name="b_" + name)
                nc.sync.dma_start(t, wap[name].rearrange("(m o) -> m o", o=1))
            return t

        def ffn(w, b, chunks, n, func, out_tag, out_dt=bf16, out_bufs=12):
            kb_n = len(chunks)
            m_n = w.shape[2] // P
            outs = []
            for mb in range(m_n):
                acc = ps.tile([P, n], f32, tag="mm", bufs=mm_bufs, name="accf")
                for kb in range(kb_n):
                    mm(acc, w[:, kb, mb * P:(mb + 1) * P], chunks[kb],
                       kb == 0, kb == kb_n - 1)
                o = sb.tile([P, n], out_dt, tag=out_tag, bufs=out_bufs,
                            name="ffn_o")
                with nc.allow_low_precision(**lp):
                    nc.scalar.activation(o, acc, func, bias=b[:, mb:mb + 1])
                outs.append(o)
            return outs

        def layer_norm(chunks, n, g, be, out_tag, out_dt=f32, out_bufs=6):
            muB = ps.tile([P, n], f32, tag="att", bufs=att_bufs, name="muB")
            for c in range(KC):
                mm(muB, onesW, chunks[c], c == 0, c == KC - 1)
            s2B = ps.tile([P, n], f32, tag="att", bufs=att_bufs, name="s2B")
            for c in range(KC):
                sq = sb.tile([P, n], bf16, tag="sq", bufs=4, name="sq")
                nc.vector.tensor_mul(sq, chunks[c], chunks[c])
                mm(s2B, onesW, sq, c == 0, c == KC - 1)
            muBs = sb.tile([P, n], bf16, tag="sq", bufs=4, name="muBs")
            with nc.allow_low_precision(**lp):
                nc.scalar.activation(muBs, muB, AF.Copy, scale=1.0 / H)
            t1 = sb.tile([P, n], f32, tag="tmp2", bufs=4, name="t1")
            nc.vector.tensor_mul(t1, muBs, muBs)
            varB = sb.tile([P, n], f32, tag="tmp2", bufs=4, name="varB")
            nc.vector.scalar_tensor_tensor(
                out=varB, in0=s2B, scalar=1.0 / H, in1=t1,
                op0=mybir.AluOpType.mult, op1=mybir.AluOpType.subtract)
            sdB = sb.tile([P, n], f32, tag="tmp2", bufs=4, name="sdB")
            nc.scalar.activation(sdB, varB, AF.Sqrt, bias=epsln[:, 0:1])
            rstdB = sb.tile([P, n], f32, tag="tmp2", bufs=4, name="rstdB")
            nc.vector.reciprocal(rstdB, sdB)
            outs = []
            for c in range(KC):
                t = sb.tile([P, n], f32, tag="tmp2", bufs=4, name="lnt")
                nc.vector.tensor_sub(t, chunks[c], muBs)
                nc.vector.tensor_mul(t, t, rstdB)
                o = sb.tile([P, n], out_dt, tag=out_tag, bufs=out_bufs,
                            name="ln_o")
                with nc.allow_low_precision(**lp):
                    nc.scalar.activation(o, t, AF.Identity,
                                         bias=be[:, c:c + 1],
                                         scale=g[:, c:c + 1])
                outs.append(o)
            return outs

        def load_fm(src_dram, t0, n, identb_, out_tag="x", out_bufs=38):
            """Load [n,H] bf16 token-major rows, PE-transpose to fm chunks."""
            blocks = []
            for b in range(n // P):
                blk = sb.tile([P, H], bf16, tag="tmin2", bufs=10, name="blk")
                nc.sync.dma_start(blk, src_dram[t0 + b * P: t0 + (b + 1) * P, :])
                blocks.append(blk)
            outs = []
            for c in range(KC):
                xc = sb.tile([P, n], bf16, tag=out_tag, bufs=out_bufs,
                             name="xc")
                for b in range(n // P):
                    tr = ps.tile([P, P], bf16, tag="mm", bufs=mm_bufs,
                                 name="tr")
                    nc.tensor.transpose(
                        tr, blocks[b][:, c * P:(c + 1) * P], identb_)
                    copy_alt(xc[:, b * P:(b + 1) * P], tr)
                outs.append(xc)
            return outs

        return (mm, copy_alt, load_w, load_v, ffn, layer_norm, load_fm,
                onesW, onesbc, epsln)

    # ============ Context 1: support + prototypes + adaptation ============
    with tile.TileContext(nc) as tc:
        with tc.tile_pool(name="cp1", bufs=1) as cpl, \
             tc.tile_pool(name="sb1", bufs=2) as sb, \
             tc.tile_pool(name="ps1", bufs=2, space="PSUM") as ps, \
             tc.tile_pool(name="dram1", bufs=1, space="DRAM") as dram:
            (mm, copy_alt, load_w, load_v, ffn, layer_norm, load_fm,
             onesW, onesbc, epsln) = make_ops(tc, sb, ps, cpl, att_bufs=5)
            ident = cpl.tile([P, P], f32, tag="ident", name="ident")
            make_identity(nc, ident)
            identb = cpl.tile([P, P], bf16, tag="identb", name="identb")
            make_identity(nc, identb)

            aw_W1 = load_w("aw_W1")
            aw_W2 = load_w("aw_W2")
            aw_b1 = load_v("aw_b1"); aw_b2 = load_v("aw_b2")
            aw_g = load_v("aw_g"); aw_be = load_v("aw_be")
            ad_W1 = load_w("ad_W1")
            ad_W2 = load_w("ad_W2")
            ad_b1 = load_v("ad_b1"); ad_b2 = load_v("ad_b2")
            pfm = [cpl.tile([P, NEX], f32, tag=f"pfm{c}", name=f"pfm{c}")
                   for c in range(KC)]

            # ---- support encode + masked mean ----
            for st in range(TOKS // TS_A):
                t0 = st * TS_A
                xs = load_fm(se, t0, TS_A, identb)
                h1 = ffn(aw_W1, aw_b1, xs, TS_A, AF.Relu, "h1")
                hh = ffn(aw_W2, aw_b2, h1, TS_A, AF.Identity, "inp")
                aware = layer_norm(hh, TS_A, aw_g, aw_be, "aw_s", out_bufs=6)
                mwrowf = sb.tile([1, TS_A], f32, tag="rowsb", bufs=8,
                                 name="mwrf")
                nc.sync.dma_start(mwrowf, mw[:, t0:t0 + TS_A])
                mwrow = sb.tile([1, TS_A], bf16, tag="rowsb", bufs=8,
                                name="mwr")
                with nc.allow_low_precision(**lp):
                    nc.scalar.activation(mwrow, mwrowf, AF.Copy)
                mwB = ps.tile([P, TS_A], f32, tag="att", bufs=5, name="mwB")
                mm(mwB, onesbc, mwrow, True, True)
                for c in range(KC):
                    wt = sb.tile([P, TS_A], f32, tag="tmp2", bufs=4, name="wt")
                    nc.vector.tensor_mul(wt, aware[c], mwB)
                    for e in range(TS_A // SS):
                        eg = st * (TS_A // SS) + e
                        nc.vector.reduce_sum(
                            pfm[c][:, eg:eg + 1],
                            wt[:, e * SS:(e + 1) * SS], axis=AX.X)

            # ---- AllGather prototypes ----
            protos_local = dram.tile([NEX, H], f32, name="protos_local")
            plr = protos_local.rearrange("e (c p) -> p c e", p=P)
            for c in range(KC):
                nc.sync.dma_start(plr[:, c, :], pfm[c])
            protos_all = dram.tile([SB, H], f32, name="protos_all")
            nc.gpsimd.collective_compute(
                "AllGather", mybir.AluOpType.bypass,
                ins=[protos_local.opt()], outs=[protos_all.opt()],
                replica_groups=[list(range(NCORES))])
            nc.sync.dma_start(protos_o, protos_all)

            # ---- normalized prototypes (on-chip) ----
            ptf = cpl.tile([SB, H], f32, tag="ptf", name="ptf")
            nc.sync.dma_start(ptf, protos_all)
            pt = cpl.tile([SB, H], bf16, tag="pt", name="pt")
            with nc.allow_low_precision(**lp):
                nc.scalar.activation(pt, ptf, AF.Copy)
            psq = sb.tile([SB, H], f32, tag="tmin", bufs=2, name="psq")
            nc.vector.tensor_mul(psq, ptf, ptf)
            n2 = sb.tile([SB, 1], f32, tag="small", bufs=4, name="n2")
            nc.vector.reduce_sum(n2, psq, axis=AX.X)
            nrm = sb.tile([SB, 1], f32, tag="small", bufs=4, name="nrm")
            nc.scalar.activation(nrm, n2, AF.Sqrt)
            nc.vector.tensor_scalar_max(nrm, nrm, 1e-8)
            rin = sb.tile([SB, 1], f32, tag="small", bufs=4, name="rin")
            nc.vector.reciprocal(rin, nrm)
            pn_tm = sb.tile([SB, H], f32, tag="tmin", bufs=2, name="pn_tm")
            nc.vector.tensor_scalar_mul(pn_tm, ptf, rin)
            pnf = []
            ptfm = []
            for c in range(KC):
                tr = ps.tile([P, 32], f32, tag="mm", bufs=3, name="trp")
                nc.tensor.transpose(tr, pn_tm[:, c * P:(c + 1) * P],
                                    ident[0:SB, 0:SB])
                pc = cpl.tile([P, SB], bf16, tag=f"pnf{c}", name=f"pnf{c}")
                with nc.allow_low_precision(**lp):
                    copy_alt(pc, tr)
                pnf.append(pc)
                tr2 = ps.tile([P, 32], f32, tag="mm", bufs=3, name="trp2")
                nc.tensor.transpose(tr2, ptf[:, c * P:(c + 1) * P],
                                    ident[0:SB, 0:SB])
                pf = cpl.tile([P, SB], bf16, tag=f"ptfm{c}", name=f"ptfm{c}")
                with nc.allow_low_precision(**lp):
                    copy_alt(pf, tr2)
                ptfm.append(pf)
            # PW1 = protos @ ad_W1 in proto-major [32, 768] layout,
            # zero-padded to K=128 so the fused W1 group has uniform K
            pw1tm = cpl.tile([P, H], bf16, tag="pw1tm", name="pw1tm")
            nc.vector.memset(pw1tm, 0.0)
            for half in range(2):
                acc = ps.tile([SB, H // 2], f32, tag="mm", bufs=3,
                              name="pw1a")
                for kb in range(KC):
                    mm(acc, ptfm[kb],
                       ad_W1[:, kb, half * (H // 2):(half + 1) * (H // 2)],
                       kb == 0, kb == KC - 1)
                with nc.allow_low_precision(**lp):
                    copy_alt(
                        pw1tm[0:SB, half * (H // 2):(half + 1) * (H // 2)],
                        acc)

            # ---- adaptation: two tiles interleaved ----
            def adapt_step(x):
                acc = sb.tile([P, TS_A], f32, tag="ssacc", bufs=4, name="sacc")
                sq0 = sb.tile([P, TS_A], bf16, tag="sq", bufs=4, name="sq0")
                nc.vector.tensor_mul(sq0, x[0], x[0])
                sq1 = sb.tile([P, TS_A], bf16, tag="sq", bufs=4, name="sq1")
                nc.vector.tensor_mul(sq1, x[1], x[1])
                nc.vector.tensor_add(acc, sq0, sq1)
                for c in range(2, KC):
                    sq = sb.tile([P, TS_A], bf16, tag="sq", bufs=4, name="sq")
                    nc.vector.tensor_mul(sq, x[c], x[c])
                    nc.vector.tensor_add(acc, acc, sq)
                ssr = sb.tile([P, TS_A], f32, tag="ssacc", bufs=4, name="ssr")
                nc.gpsimd.partition_all_reduce(
                    ssr, acc, channels=P, reduce_op=bass_isa.ReduceOp.add)
                simsT = ps.tile([32, TS_A], f32, tag="att", bufs=5,
                                name="simsT")
                for c in range(KC):
                    mm(simsT, pnf[c], x[c], c == 0, c == KC - 1)
                nrmxB = sb.tile([32, TS_A], f32, tag="attsb", bufs=8,
                                name="nrmxB")
                nc.scalar.activation(nrmxB, ssr[0:SB, :], AF.Sqrt)
                rnoBs = sb.tile([32, TS_A], f32, tag="attsb", bufs=8,
                                name="rnoBs")
                nc.vector.reciprocal(rnoBs, nrmxB)
                ssc = sb.tile([32, TS_A], f32, tag="attsb", bufs=8, name="ssc")
                nc.vector.tensor_mul(ssc, simsT, rnoBs)
                E = sb.tile([32, TS_A], bf16, tag="attsb", bufs=8, name="E")
                with nc.allow_low_precision(**lp):
                    nc.scalar.activation(E, ssc, AF.Exp)
                ZB = ps.tile([32, TS_A], f32, tag="att", bufs=5, name="ZB")
                mm(ZB, onesW[0:SB, 0:SB], E, True, True)
                rzc = sb.tile([32, TS_A], f32, tag="attsb", bufs=8, name="rzc")
                nc.vector.reciprocal(rzc, ZB)
                wgt = sb.tile([P, TS_A], bf16, tag="wgt128", bufs=4,
                              name="wgt")
                for pb in range(SB, P, SB):
                    nc.vector.memset(wgt[pb:pb + SB, :], 0.0)
                with nc.allow_low_precision(**lp):
                    nc.vector.tensor_mul(wgt[0:SB, :], E, rzc)
                # h1 = relu(x@W1 + wgt.T@(protos@W1) + b1): x-part streams
                # without waiting on the softmax chain.
                h1 = []
                for mb in range(KC):
                    acc = ps.tile([P, TS_A], f32, tag="mm", bufs=3,
                                  name="acch")
                    for kb in range(KC):
                        mm(acc, ad_W1[:, kb, mb * P:(mb + 1) * P], x[kb],
                           kb == 0, False)
                    mm(acc, pw1tm[:, mb * P:(mb + 1) * P], wgt, False, True)
                    o = sb.tile([P, TS_A], bf16, tag="h1", bufs=12,
                                name="h1f")
                    with nc.allow_low_precision(**lp):
                        nc.scalar.activation(o, acc, AF.Relu,
                                             bias=ad_b1[:, mb:mb + 1])
                    h1.append(o)
                return ffn(ad_W2, ad_b2, h1, TS_A, AF.Identity, "x",
                           out_bufs=38)

            for pair in range(NT_A // 2):
                xs2 = []
                for t in range(2):
                    ti = pair * 2 + t
                    t0 = ti * TS_A
                    xs2.append(load_fm(qe, t0, TS_A, identb))
                for _ in range(ADAPT_STEPS):
                    xs2 = [adapt_step(xs2[0]), adapt_step(xs2[1])]
                for t in range(2):
                    ti = pair * 2 + t
                    t0 = ti * TS_A
                    x = xs2[t]
                    for c in range(KC):
                        nc.sync.dma_start(afm[:, c, t0:t0 + TS_A], x[c])
                    for b in range(TS_A // P):
                        atm = sb.tile([P, H], f32, tag="tmout", bufs=4,
                                      name="atm")
                        for c in range(KC):
                            tr = ps.tile([P, P], bf16, tag="mm", bufs=3,
                                         name="tra")
                            nc.tensor.transpose(
                                tr, x[c][:, b * P:(b + 1) * P], identb)
                            copy_alt(atm[:, c * P:(c + 1) * P], tr)
                        nc.sync.dma_start(
                            adapted_o[t0 + b * P: t0 + (b + 1) * P, :], atm)

    # ============ Context 2: encoders + LN + meta classifier ============
    with tile.TileContext(nc) as tc:
        with tc.tile_pool(name="cp3", bufs=1) as cpl, \
             tc.tile_pool(name="sb3", bufs=2) as sb, \
             tc.tile_pool(name="ps3", bufs=2, space="PSUM") as ps:
            (mm, copy_alt, load_w, load_v, ffn, layer_norm, load_fm,
             onesW, onesbc, epsln) = make_ops(tc, sb, ps, cpl)
            identb = cpl.tile([P, P], bf16, tag="identb", name="identb")
            make_identity(nc, identb)

            aw_W1f = load_w("aw_W1", "aw_W1f")
            aw_W2f = load_w("aw_W2", "aw_W2f")
            aw_b1 = load_v("aw_b1"); aw_b2 = load_v("aw_b2")
            aw_g = load_v("aw_g"); aw_be = load_v("aw_be")
            ct_W1 = load_w("ct_W1")
            ct_W2 = load_w("ct_W2")
            ct_b1 = load_v("ct_b1"); ct_b2 = load_v("ct_b2")
            ct_g = load_v("ct_g"); ct_be = load_v("ct_be")
            mc_W1t = load_w("mc_W1")
            mc_W2t = cpl.tile([P, KC, 64], bf16, tag="w_mc_W2", name="w_mc_W2")
            nc.sync.dma_start(mc_W2t,
                              wap["mc_W2"].rearrange("(k p) m -> p k m", p=P))
            mc_W3t = cpl.tile([64, 3], bf16, tag="w_mc_W3", name="w_mc_W3")
            nc.sync.dma_start(mc_W3t, wap["mc_W3"])
            mc_b1 = load_v("mc_b1"); mc_b2 = load_v("mc_b2")
            mc_b3 = load_v("mc_b3")

            for tj in range(NT_F):
                t0 = tj * TS_F
                adT = []
                for c in range(KC):
                    a = sb.tile([P, TS_F], bf16, tag="x", bufs=14, name="adT")
                    nc.sync.dma_start(a, afm[:, c, t0:t0 + TS_F])
                    adT.append(a)
                h1a = ffn(aw_W1f, aw_b1, adT, TS_F, AF.Relu, "h1")
                hha = ffn(aw_W2f, aw_b2, h1a, TS_F, AF.Identity, "inp")
                qa = layer_norm(hha, TS_F, aw_g, aw_be, "qln",
                                out_dt=bf16, out_bufs=26)
                h1c = ffn(ct_W1, ct_b1, adT, TS_F, AF.Relu, "h1")
                hhc = ffn(ct_W2, ct_b2, h1c, TS_F, AF.Identity, "inp")
                qc = layer_norm(hhc, TS_F, ct_g, ct_be, "qln",
                                out_dt=bf16, out_bufs=26)

                mh1 = []
                for mb in range(KC):
                    acc = ps.tile([P, TS_F], f32, tag="mm", bufs=3,
                                  name="accm")
                    for kb in range(12):
                        src = qa[kb] if kb < KC else qc[kb - KC]
                        mm(acc, mc_W1t[:, kb, mb * P:(mb + 1) * P], src,
                           kb == 0, kb == 11)
                    o = sb.tile([P, TS_F], bf16, tag="h1", bufs=12, name="mh1")
                    with nc.allow_low_precision(**lp):
                        nc.scalar.activation(o, acc, AF.Relu,
                                             bias=mc_b1[:, mb:mb + 1])
                    mh1.append(o)
                acc2 = ps.tile([64, TS_F], f32, tag="mm", bufs=3, name="acc2")
                for kb in range(KC):
                    mm(acc2, mc_W2t[:, kb, 0:64], mh1[kb], kb == 0,
                       kb == KC - 1)
                h2 = sb.tile([64, TS_F], bf16, tag="h2", bufs=4, name="h2")
                with nc.allow_low_precision(**lp):
                    nc.scalar.activation(h2, acc2, AF.Relu,
                                         bias=mc_b2[:, 0:1])
                acc3 = ps.tile([3, TS_F], f32, tag="row", bufs=2, name="acc3")
                mm(acc3, mc_W3t, h2, True, True)
                pr = sb.tile([3, TS_F], f32, tag="pr", bufs=4, name="pr")
                nc.scalar.activation(pr, acc3, AF.Identity,
                                     bias=mc_b3[:, 0:1])
                nc.sync.dma_start(
                    preds_o[t0:t0 + TS_F, :].rearrange("t r -> r t"), pr)

                for br, qch in enumerate((qa, qc)):
                    for b in range(TS_F // P):
                        half = sb.tile([P, H], f32, tag="tmout", bufs=12,
                                       name="half")
                        for c in range(KC):
                            tr = ps.tile([P, P], bf16, tag="mm", bufs=3,
                                         name="trq")
                            nc.tensor.transpose(
                                tr, qch[c][:, b * P:(b + 1) * P], identb)
                            copy_alt(half[:, c * P:(c + 1) * P], tr)
                        nc.sync.dma_start(
                            qcomb_o[t0 + b * P: t0 + (b + 1) * P,
                                    br * H:(br + 1) * H], half)

    nc.compile()
    return nc


def kernel(support_embeddings, support_labels, query_embeddings, params):
    global LAST_EXEC_NS
    import ml_dtypes
    from concourse import bass_utils

    se = np.ascontiguousarray(np.asarray(support_embeddings), dtype=np.float32)
    sl = np.asarray(support_labels)
    qe = np.ascontiguousarray(np.asarray(query_embeddings), dtype=np.float32)
    pr = {k: np.ascontiguousarray(np.asarray(v), dtype=np.float32)
          for k, v in params.items()}

    mask = (sl > 0).astype(np.float32)
    denom = np.maximum(mask.sum(-1, keepdims=True), 1.0)
    maskw = (mask / denom).astype(np.float32)          # [32, 256]

    if "nc" not in _CACHE:
        _CACHE["nc"] = _build()
    nc = _CACHE["nc"]

    in_maps = []
    for i in range(NCORES):
        m = {
            "qebf": np.ascontiguousarray(
                qe[i * 8:(i + 1) * 8].reshape(TOKQ, H)).astype(
                    ml_dtypes.bfloat16),
            "sebf": np.ascontiguousarray(
                se[i * NEX:(i + 1) * NEX].reshape(TOKS, H)).astype(
                    ml_dtypes.bfloat16),
            "mw": np.ascontiguousarray(
                maskw[i * NEX:(i + 1) * NEX].reshape(1, TOKS)),
        }
        for k in W2D:
            m[k + "bf"] = pr[k].astype(ml_dtypes.bfloat16)
        for k in V1D:
            m[k] = pr[k]
        in_maps.append(m)

    res = bass_utils.run_bass_kernel_spmd(nc, in_maps,
                                          core_ids=list(range(NCORES)))
    LAST_EXEC_NS = res.exec_time_ns
    outs = res.results

    preds = np.concatenate(
        [o["preds_o"].reshape(8, QS, 3) for o in outs], axis=0)
    adapted = np.concatenate(
        [o["adapted_o"].reshape(8, QS, H) for o in outs], axis=0)
    qcomb = np.concatenate(
        [o["qcomb_o"].reshape(8, QS, 2 * H) for o in outs], axis=0)
    protos = outs[0]["protos_o"]
    return preds, adapted, protos, qcomb
